# revision 17
# baseline (speedup 1.0000x reference)
"""AttnDecoderGRU step — 8-core Trainium2 Bass kernel.

Sharding (tensor-parallel, 3 AllReduces):
  - GRU: gate-dim sharded. Core k computes gate rows {g*1536 + k*192 .. +192}
    for g in {r,z,n} -> h_newT slice [192, 256].
  - Attention: H-sharded. Core k's scores partial uses its h-slice of h_new
    and enc; AllReduce #1 sums score partials [128s, 256b]. Softmax is
    replicated; context for the core's h-slice is then fully local.
  - Concat proj: contraction(2H)-sharded; AllReduce #2 sums pre-tanh
    partials [1536, 256] (transposed layout).
  - Output proj: vocab-sharded (4000 rows of W_out per core); sharded softmax
    via AllReduce #3 of the per-row exp-sums [256].
Matmul operands are fp16 (fp32 accumulate in PSUM, fp16 gets FWL fast
weight loads and full PE rate); gather/softmax/elementwise/collectives f32.

Measured (8x NC_v3 via axon/PJRT, full-input staging excluded from err):
  probs absmax-rel err 4.0e-3 (l2 6.4e-4), h_new absmax-rel 1.7e-4.
Per-core budget estimate (cost model; NTFF profiling unavailable in this
client): DMA reads ~44MB (~125us floor), PE ~120-150us (outproj 42us,
attention per-batch-column matmuls ~60us, GRU 13us), 3 AllReduces
~90-110us partially exposed => ~250-350us/step. Next optimizations:
(1) split AllReduce #2 by batch halves and overlap with outproj start,
(2) tile_position col-packing (4x) for the 1024 attention LDW+matmul
pairs, (3) prefetch encC during scores phase.
"""

import numpy as np
import ml_dtypes
from contextlib import ExitStack

H, V, B, S, C = 1536, 32000, 256, 128, 8
HK = H // C          # 192  per-core h slice
GK = 3 * HK          # 576  per-core gate rows
VK = V // C          # 4000 per-core vocab slice
P = 128

_CACHE = {}


def _build_nc():
    import concourse.bass as bass
    import concourse.tile as tile
    import concourse.mybir as mybir
    from concourse import bacc
    from concourse.masks import make_identity

    dt = mybir.dt
    F32, BF, I32 = dt.float32, dt.bfloat16, dt.int32
    FR = dt.float32r
    F16 = dt.float16
    AF = mybir.ActivationFunctionType
    Alu = mybir.AluOpType
    AX = mybir.AxisListType

    nc = bacc.Bacc("TRN2", target_bir_lowering=False, debug=False, num_devices=C)

    # ---- I/O ----
    ids_p = nc.declare_dram_parameter("ids32", [B], I32, isOutput=False)
    emb_p = nc.declare_dram_parameter("emb", [V, H], F32, isOutput=False)
    hk_p = nc.declare_dram_parameter("hk32", [HK, B], F32, isOutput=False)
    hT_p = nc.declare_dram_parameter("hT32", [H, B], dt.float16, isOutput=False)
    wih_p = nc.declare_dram_parameter("wih_t", [H, GK], dt.float16, isOutput=False)
    whh_p = nc.declare_dram_parameter("whh_t", [H, GK], dt.float16, isOutput=False)
    brz_p = nc.declare_dram_parameter("brz", [2 * HK], F32, isOutput=False)
    bin_p = nc.declare_dram_parameter("bin_", [HK], F32, isOutput=False)
    bhn_p = nc.declare_dram_parameter("bhn", [HK], F32, isOutput=False)
    encA_p = nc.declare_dram_parameter("encA", [HK, B, S], dt.float16, isOutput=False)  # [h,b,s]
    encC_p = nc.declare_dram_parameter("encC", [S, B, HK], dt.float16, isOutput=False)  # [s,b,h]
    wc_p = nc.declare_dram_parameter("wc_t", [2 * HK, H], dt.float16, isOutput=False)
    bc_p = nc.declare_dram_parameter("bc", [H], F32, isOutput=False)
    wo_p = nc.declare_dram_parameter("wo_t", [H, VK], dt.float16, isOutput=False)
    bo_p = nc.declare_dram_parameter("bo32", [VK], dt.float16, isOutput=False)
    hout_p = nc.declare_dram_parameter("h_out", [HK, B], F32, isOutput=True)
    probs_p = nc.declare_dram_parameter("probs", [B, VK], F32, isOutput=True)

    RG = [list(range(C))]
    NHC = H // P  # 12

    with tile.TileContext(nc) as tc, ExitStack() as ctx:
        sing = ctx.enter_context(tc.tile_pool(name="sing", bufs=1))
        dramp = ctx.enter_context(tc.tile_pool(name="dram", bufs=1, space="DRAM"))

        gruw_cm = tc.tile_pool(name="gruw", bufs=1)
        gruw = gruw_cm.__enter__()

        # ---- embedding gather first (indirect DMA must carry few waits) ----
        xgs = []
        for bb in range(2):
            idt = gruw.tile([P, 1], I32, tag=f"idt{bb}", name=f"idt{bb}")
            nc.gpsimd.dma_start(
                out=idt[:], in_=ids_p[bb * P:(bb + 1) * P].rearrange("(p o) -> p o", o=1))
            xg = gruw.tile([P, H], F32, tag=f"xg{bb}", name=f"xg{bb}")
            nc.gpsimd.indirect_dma_start(
                out=xg[:], out_offset=None, in_=emb_p[:, :],
                in_offset=bass.IndirectOffsetOnAxis(ap=idt[:, :1], axis=0))
            xgs.append(xg)

        ident = sing.tile([P, P], F32, tag="ident")
        make_identity(nc, ident[:])

        # persistent loads
        hTr = gruw.tile([P, NHC, B], F16, tag="hTr")
        nc.gpsimd.dma_start(out=hTr[:], in_=hT_p.rearrange("(c p) b -> p c b", p=P))
        wih_s = gruw.tile([P, NHC, GK], F16, tag="wih")
        nc.gpsimd.dma_start(out=wih_s[:], in_=wih_p.rearrange("(c p) g -> p c g", p=P))
        whh_s = gruw.tile([P, NHC, GK], F16, tag="whh")
        nc.gpsimd.dma_start(out=whh_s[:], in_=whh_p.rearrange("(c p) g -> p c g", p=P))
        hk0 = sing.tile([P, B], F32, tag="hk0")
        nc.gpsimd.dma_start(out=hk0[:], in_=hk_p[0:P, :])
        hk1 = sing.tile([HK - P, B], F32, tag="hk1")
        nc.gpsimd.dma_start(out=hk1[:], in_=hk_p[P:HK, :])
        bc_s = sing.tile([P, NHC], F32, tag="bc")
        nc.gpsimd.dma_start(out=bc_s[:], in_=bc_p.rearrange("(c p) -> p c", p=P))

        # gate biases: rz chunks aligned per gate: r:(0,128),(128,64) z:(192,128),(320,64)
        rz_chunks = [(0, P), (P, HK - P), (HK, P), (HK + P, HK - P)]
        brz_t = []
        for i, (off, cnt) in enumerate(rz_chunks):
            t = sing.tile([cnt, 1], F32, tag=f"brz{i}")
            nc.gpsimd.dma_start(out=t[:], in_=brz_p[off:off + cnt].rearrange("(p o) -> p o", o=1))
            brz_t.append(t)
        n_chunks = [(0, P), (P, HK - P)]  # offsets within n-slice
        bin_t, bhn_t = [], []
        for i, (off, cnt) in enumerate(n_chunks):
            t = sing.tile([cnt, 1], F32, tag=f"bin{i}")
            nc.gpsimd.dma_start(out=t[:], in_=bin_p[off:off + cnt].rearrange("(p o) -> p o", o=1))
            bin_t.append(t)
            t2 = sing.tile([cnt, 1], F32, tag=f"bhn{i}")
            nc.gpsimd.dma_start(out=t2[:], in_=bhn_p[off:off + cnt].rearrange("(p o) -> p o", o=1))
            bhn_t.append(t2)

        # ---- transpose x -> xTr [P, NHC, B] (f32r) ----
        xTr = gruw.tile([P, NHC, B], F16, tag="xTr")
        with tc.tile_pool(name="psT1", bufs=2, space="PSUM") as psT:
            for bb in range(2):
                for hc in range(NHC):
                    tp = psT.tile([P, P], F32, tag="tp")
                    nc.tensor.transpose(out=tp[:], in_=xgs[bb][:, hc * P:(hc + 1) * P], identity=ident[:])
                    nc.vector.tensor_copy(out=xTr[:, hc, bb * P:(bb + 1) * P], in_=tp[:])

        # ---- GRU ----
        gp_cm = tc.tile_pool(name="gp", bufs=2)
        gp = gp_cm.__enter__()
        psG_cm = tc.tile_pool(name="psG", bufs=4, space="PSUM")
        psG = psG_cm.__enter__()
        rz_sb = []
        for i, (off, cnt) in enumerate(rz_chunks):
            ps = psG.tile([cnt, B], F32, tag="gps")
            for hc in range(NHC):
                nc.tensor.matmul(out=ps[:], lhsT=wih_s[:, hc, off:off + cnt], rhs=xTr[:, hc, :],
                                 start=(hc == 0), stop=False)
            for hc in range(NHC):
                nc.tensor.matmul(out=ps[:], lhsT=whh_s[:, hc, off:off + cnt], rhs=hTr[:, hc, :],
                                 start=False, stop=(hc == NHC - 1))
            g = gp.tile([cnt, B], F32, tag=f"rz{i}")
            nc.scalar.activation(out=g[:], in_=ps[:], func=AF.Sigmoid, bias=brz_t[i][:])
            rz_sb.append(g)

        hn_f, hnbf = [], []
        for i, (off, cnt) in enumerate(n_chunks):
            goff = 2 * HK + off
            ps_gi = psG.tile([cnt, B], F32, tag="gps")
            for hc in range(NHC):
                nc.tensor.matmul(out=ps_gi[:], lhsT=wih_s[:, hc, goff:goff + cnt], rhs=xTr[:, hc, :],
                                 start=(hc == 0), stop=(hc == NHC - 1))
            ps_gh = psG.tile([cnt, B], F32, tag="gps")
            for hc in range(NHC):
                nc.tensor.matmul(out=ps_gh[:], lhsT=whh_s[:, hc, goff:goff + cnt], rhs=hTr[:, hc, :],
                                 start=(hc == 0), stop=(hc == NHC - 1))
            ghn = gp.tile([cnt, B], F32, tag="ghn")
            nc.scalar.activation(out=ghn[:], in_=ps_gh[:], func=AF.Identity, bias=bhn_t[i][:])
            t1 = gp.tile([cnt, B], F32, tag="t1")
            nc.vector.tensor_mul(out=t1[:], in0=rz_sb[i][:], in1=ghn[:])  # r * (ghn+bhn)
            t2 = gp.tile([cnt, B], F32, tag="t2")
            nc.vector.tensor_add(out=t2[:], in0=t1[:], in1=ps_gi[:])
            n_t = gp.tile([cnt, B], F32, tag="nt")
            nc.scalar.activation(out=n_t[:], in_=t2[:], func=AF.Tanh, bias=bin_t[i][:])
            hk_c = hk0 if i == 0 else hk1
            d = gp.tile([cnt, B], F32, tag="d")
            nc.vector.tensor_sub(out=d[:], in0=hk_c[:], in1=n_t[:])
            zd = gp.tile([cnt, B], F32, tag="zd")
            nc.vector.tensor_mul(out=zd[:], in0=rz_sb[2 + i][:], in1=d[:])
            hn = sing.tile([cnt, B], F32, tag=f"hn{i}", name=f"hn{i}")
            nc.vector.tensor_add(out=hn[:], in0=n_t[:], in1=zd[:])
            hn_f.append(hn)
            hb = sing.tile([cnt, B], F16, tag=f"hnbf{i}")
            nc.vector.tensor_copy(out=hb[:], in_=hn[:])
            hnbf.append(hb)
            nc.gpsimd.dma_start(out=hout_p[off:off + cnt, :], in_=hn[:])

        psG_cm.__exit__(None, None, None)
        gp_cm.__exit__(None, None, None)
        gruw_cm.__exit__(None, None, None)

        # ---- scores (partial over this core's h-slice) ----
        psS_cm = tc.tile_pool(name="psS", bufs=1, space="PSUM")
        psS = psS_cm.__enter__()
        scps = psS.tile([P, B], F32, tag="scps")
        with tc.tile_pool(name="apool", bufs=2) as apool:
            for blk in range(B // 32):
                b0 = blk * 32
                eA0 = apool.tile([P, 32, S], F16, tag="eA0")
                nc.gpsimd.dma_start(out=eA0[:], in_=encA_p[0:P, b0:b0 + 32, :])
                eA1 = apool.tile([HK - P, 32, S], F16, tag="eA1")
                nc.gpsimd.dma_start(out=eA1[:], in_=encA_p[P:HK, b0:b0 + 32, :])
                for b in range(32):
                    ba = b0 + b
                    nc.tensor.matmul(out=scps[:, ba:ba + 1], lhsT=eA0[:, b, :],
                                     rhs=hnbf[0][:, ba:ba + 1], start=True, stop=False)
                    nc.tensor.matmul(out=scps[:, ba:ba + 1], lhsT=eA1[:, b, :],
                                     rhs=hnbf[1][:, ba:ba + 1], start=False, stop=True)
        sc_sb = sing.tile([P, B], F32, tag="sc_sb")
        nc.vector.tensor_copy(out=sc_sb[:], in_=scps[:])
        sc_in = dramp.tile([P, B], F32, tag="sc_in")
        nc.gpsimd.dma_start(out=sc_in[:], in_=sc_sb[:])
        sc_out = dramp.tile([P, B], F32, tag="sc_out", addr_space="Shared")
        nc.gpsimd.collective_compute(
            "AllReduce", Alu.add, replica_groups=RG, ins=[sc_in[:].opt()], outs=[sc_out[:].opt()])
        scf = sing.tile([P, B], F32, tag="scf")
        nc.gpsimd.dma_start(out=scf[:], in_=sc_out[:])

        # ---- softmax over s (free dim after transpose) ----
        attbf = sing.tile([P, B], F16, tag="attbf")  # [s, b]
        with tc.tile_pool(name="smx", bufs=4) as smx, \
             tc.tile_pool(name="psT2", bufs=2, space="PSUM") as psT:
            for bb in range(2):
                tp = psT.tile([P, P], F32, tag="tp")
                nc.tensor.transpose(out=tp[:], in_=scf[:, bb * P:(bb + 1) * P], identity=ident[:])
                scb = smx.tile([P, P], F32, tag="scb")
                nc.vector.tensor_copy(out=scb[:], in_=tp[:])
                mx = smx.tile([P, 1], F32, tag="mx")
                nc.vector.reduce_max(out=mx[:], in_=scb[:], axis=AX.X)
                nmx = smx.tile([P, 1], F32, tag="nmx")
                nc.vector.tensor_scalar_mul(nmx[:], scb_in0 := mx[:], -1.0)
                ex = smx.tile([P, P], F32, tag="ex")
                nc.scalar.activation(out=ex[:], in_=scb[:], func=AF.Exp, bias=nmx[:])
                sm = smx.tile([P, 1], F32, tag="sm")
                nc.vector.reduce_sum(out=sm[:], in_=ex[:], axis=AX.X)
                ri = smx.tile([P, 1], F32, tag="ri")
                nc.vector.reciprocal(out=ri[:], in_=sm[:])
                at = smx.tile([P, P], F32, tag="at")
                nc.vector.tensor_scalar_mul(at[:], ex[:], ri[:])
                tp2 = psT.tile([P, P], F32, tag="tp")
                nc.tensor.transpose(out=tp2[:], in_=at[:], identity=ident[:])
                nc.vector.tensor_copy(out=attbf[:, bb * P:(bb + 1) * P], in_=tp2[:])

        # ---- context (local, h-sliced) ----
        cx0 = psS.tile([P, B], F32, tag="cx0")
        cx1 = psS.tile([HK - P, B], F32, tag="cx1")
        with tc.tile_pool(name="cpool", bufs=2) as cpool:
            for blk in range(B // 32):
                b0 = blk * 32
                eC = cpool.tile([S, 32, HK], F16, tag="eC")
                nc.gpsimd.dma_start(out=eC[:], in_=encC_p[:, b0:b0 + 32, :])
                for b in range(32):
                    ba = b0 + b
                    nc.tensor.matmul(out=cx0[:, ba:ba + 1], lhsT=eC[:, b, 0:P],
                                     rhs=attbf[:, ba:ba + 1], start=True, stop=True)
                    nc.tensor.matmul(out=cx1[:, ba:ba + 1], lhsT=eC[:, b, P:HK],
                                     rhs=attbf[:, ba:ba + 1], start=True, stop=True)
        ctb0 = sing.tile([P, B], F16, tag="ctb0")
        nc.vector.tensor_copy(out=ctb0[:], in_=cx0[:])
        ctb1 = sing.tile([HK - P, B], F16, tag="ctb1")
        nc.vector.tensor_copy(out=ctb1[:], in_=cx1[:])
        psS_cm.__exit__(None, None, None)

        # cat rows: [hn(192); ctx(192)] -> 3 chunks of 128 (partition-shift via DMA)
        cat0 = sing.tile([P, B], F16, tag="cat0")
        nc.vector.tensor_copy(out=cat0[:], in_=hn_f[0][:])
        cat1 = sing.tile([P, B], F16, tag="cat1")
        nc.vector.tensor_copy(out=cat1[0:HK - P, :], in_=hn_f[1][:])
        nc.gpsimd.dma_start(out=cat1[HK - P:P, :], in_=ctb0[0:2 * P - HK, :])
        cat2 = sing.tile([P, B], F16, tag="cat2")
        nc.gpsimd.dma_start(out=cat2[0:HK - P, :], in_=ctb0[2 * P - HK:P, :])
        nc.gpsimd.dma_start(out=cat2[HK - P:P, :], in_=ctb1[:])
        cats = [cat0, cat1, cat2]

        # ---- concat proj (partial) + AllReduce + tanh -> cobf chunks ----
        cp_in = dramp.tile([NHC, P, B], F32, tag="cp_in")
        cp_out = dramp.tile([NHC, P, B], F32, tag="cp_out", addr_space="Shared")
        with tc.tile_pool(name="cc", bufs=4) as cc, \
             tc.tile_pool(name="wcp", bufs=1) as wcp, \
             tc.tile_pool(name="psC", bufs=4, space="PSUM") as psC:
            wc_s = wcp.tile([P, 3, H], F16, tag="wc")
            nc.gpsimd.dma_start(out=wc_s[:], in_=wc_p.rearrange("(c p) m -> p c m", p=P))
            for mc in range(NHC):
                cps = psC.tile([P, B], F32, tag="cps")
                for kc in range(3):
                    nc.tensor.matmul(out=cps[:], lhsT=wc_s[:, kc, mc * P:(mc + 1) * P],
                                     rhs=cats[kc][:], start=(kc == 0), stop=(kc == 2))
                csb = cc.tile([P, B], F32, tag="csb")
                nc.vector.tensor_copy(out=csb[:], in_=cps[:])
                nc.gpsimd.dma_start(out=cp_in[mc], in_=csb[:])
        nc.gpsimd.collective_compute(
            "AllReduce", Alu.add, replica_groups=RG, ins=[cp_in[:].opt()], outs=[cp_out[:].opt()])
        cobf = []
        with tc.tile_pool(name="co", bufs=4) as co:
            for mc in range(NHC):
                cof = co.tile([P, B], F32, tag="cof")
                nc.gpsimd.dma_start(out=cof[:], in_=cp_out[mc])
                cb = sing.tile([P, B], F16, tag=f"cobf{mc}", name=f"cobf{mc}")
                nc.scalar.activation(out=cb[:], in_=cof[:], func=AF.Tanh, bias=bc_s[:, mc:mc + 1])
                cobf.append(cb)

        # ---- output projection + exp (vocab shard) ----
        ones_f = sing.tile([1, P], F32, tag="ones_f")
        nc.vector.memset(ones_f[:], 1.0)
        ones1 = sing.tile([1, P], F16, tag="ones1")
        nc.vector.tensor_copy(out=ones1[:], in_=ones_f[:])
        nts = [(i * 512, 512) for i in range(7)] + [(3584, VK - 3584)]
        dnt = [sing.tile([P, len(nts)], F32, tag=f"dnt{bb}", name=f"dnt{bb}") for bb in range(2)]
        ex_tiles = {}
        expool = ctx.enter_context(tc.tile_pool(name="expool", bufs=2 * len(nts)))
        with tc.tile_pool(name="wpool", bufs=2 * NHC + 2) as wpool, \
             tc.tile_pool(name="bpool", bufs=2) as bpool, \
             tc.tile_pool(name="psO", bufs=4, space="PSUM") as psO:
            for ni, (noff, ncnt) in enumerate(nts):
                wos = []
                for hc in range(NHC):
                    wo = wpool.tile([P, 512], F16, tag="wo")
                    nc.gpsimd.dma_start(out=wo[:, :ncnt], in_=wo_p[hc * P:(hc + 1) * P, noff:noff + ncnt])
                    wos.append(wo)
                bo_t = bpool.tile([1, 512], F16, tag="bo")
                nc.gpsimd.dma_start(out=bo_t[:, :ncnt], in_=bo_p[noff:noff + ncnt].rearrange("(o n) -> o n", o=1))
                for bb in range(2):
                    lg = psO.tile([P, 512], F32, tag="lg")
                    nc.tensor.matmul(out=lg[:, :ncnt], lhsT=ones1[:], rhs=bo_t[:, :ncnt],
                                     start=True, stop=False)
                    for hc in range(NHC):
                        nc.tensor.matmul(out=lg[:, :ncnt], lhsT=cobf[hc][:, bb * P:(bb + 1) * P],
                                         rhs=wos[hc][:, :ncnt], start=False, stop=(hc == NHC - 1))
                    ex = expool.tile([P, 512], F32, tag="ex")
                    nc.scalar.activation(out=ex[:, :ncnt], in_=lg[:, :ncnt], func=AF.Exp,
                                         accum_out=dnt[bb][:, ni:ni + 1])
                    ex_tiles[(bb, ni)] = ex

        # ---- sharded softmax denom: AllReduce, then scale + write out ----
        dn_in = dramp.tile([2, P], F32, tag="dn_in")
        dn_out = dramp.tile([2, P], F32, tag="dn_out", addr_space="Shared")
        with tc.tile_pool(name="dn", bufs=4) as dnp:
            for bb in range(2):
                ds = dnp.tile([P, 1], F32, tag="ds")
                nc.vector.reduce_sum(out=ds[:], in_=dnt[bb][:], axis=AX.X)
                nc.gpsimd.dma_start(out=dn_in[bb], in_=ds[:])
            nc.gpsimd.collective_compute(
                "AllReduce", Alu.add, replica_groups=RG, ins=[dn_in[:].opt()], outs=[dn_out[:].opt()])
            for bb in range(2):
                dr = dnp.tile([P, 1], F32, tag="dr")
                nc.gpsimd.dma_start(out=dr[:], in_=dn_out[bb])
                ri = dnp.tile([P, 1], F32, tag="rib")
                nc.vector.reciprocal(out=ri[:], in_=dr[:])
                for ni, (noff, ncnt) in enumerate(nts):
                    ex = ex_tiles[(bb, ni)]
                    nc.vector.tensor_scalar_mul(ex[:, :ncnt], ex[:, :ncnt], ri[:])
                    nc.gpsimd.dma_start(out=probs_p[bb * P:(bb + 1) * P, noff:noff + ncnt],
                                        in_=ex[:, :ncnt])
    if not nc.is_finalized():
        nc.finalize()
    return nc


def _prep_inputs(input_step, last_hidden, encoder_outputs, emb_table,
                 W_ih, W_hh, b_ih, b_hh, W_concat, b_concat, W_out, b_out):
    bf16 = ml_dtypes.bfloat16
    ids32 = np.ascontiguousarray(input_step[0].astype(np.int32))
    hT = np.ascontiguousarray(last_hidden[0].T.astype(np.float32))          # [H, B]
    enc = encoder_outputs.astype(np.float32)                                # [S, B, H]
    encA_full = np.ascontiguousarray(enc.transpose(2, 1, 0)).astype(np.float16)  # [H, B, S]
    enc_bf = enc.astype(np.float16)                                         # [S, B, H]
    emb = np.ascontiguousarray(emb_table.astype(np.float32))
    bsum = (b_ih + b_hh).astype(np.float32)

    in_maps = []
    for k in range(C):
        hs = slice(k * HK, (k + 1) * HK)
        gate_rows = np.concatenate([np.arange(g * H + k * HK, g * H + (k + 1) * HK) for g in range(3)])
        wih_k = np.ascontiguousarray(W_ih[gate_rows].T.astype(np.float16))  # [H, GK]
        whh_k = np.ascontiguousarray(W_hh[gate_rows].T.astype(np.float16))
        brz = np.ascontiguousarray(bsum[gate_rows[:2 * HK]])                # r,z slices
        bin_ = np.ascontiguousarray(b_ih[2 * H + k * HK:2 * H + (k + 1) * HK].astype(np.float32))
        bhn = np.ascontiguousarray(b_hh[2 * H + k * HK:2 * H + (k + 1) * HK].astype(np.float32))
        ccols = np.concatenate([np.arange(k * HK, (k + 1) * HK), np.arange(H + k * HK, H + (k + 1) * HK)])
        wc_k = np.ascontiguousarray(W_concat[:, ccols].T.astype(np.float16))    # [2HK, H]
        wo_k = np.ascontiguousarray(W_out[k * VK:(k + 1) * VK].T.astype(np.float16))  # [H, VK]
        in_maps.append({
            "ids32": ids32,
            "emb": emb,
            "hk32": np.ascontiguousarray(hT[hs]),
            "hT32": hT.astype(np.float16),
            "wih_t": wih_k,
            "whh_t": whh_k,
            "brz": brz,
            "bin_": bin_,
            "bhn": bhn,
            "encA": np.ascontiguousarray(encA_full[hs]),                    # [HK, B, S]
            "encC": np.ascontiguousarray(enc_bf[:, :, hs]),                 # [S, B, HK]
            "wc_t": wc_k,
            "bc": b_concat.astype(np.float32),
            "wo_t": wo_k,
            "bo32": b_out[k * VK:(k + 1) * VK].astype(np.float16),
        })
    return in_maps


def kernel(**inputs):
    res = _run(inputs, trace=False)
    probs = np.concatenate([res.results[k]["probs"] for k in range(C)], axis=1)  # [B, V]
    hT_new = np.concatenate([res.results[k]["h_out"] for k in range(C)], axis=0)  # [H, B]
    h_new = np.ascontiguousarray(hT_new.T)[None]                                  # [1, B, H]
    return probs.astype(np.float32), h_new.astype(np.float32)


def _run(inputs, trace=False):
    from concourse.bass_utils import run_bass_kernel_spmd
    inputs = {k: np.asarray(v) for k, v in inputs.items()}
    in_maps = _prep_inputs(**inputs)
    if "nc" not in _CACHE:
        _CACHE["nc"] = _build_nc()
    return run_bass_kernel_spmd(_CACHE["nc"], in_maps, list(range(C)), trace=trace)


def kernel_traced(**inputs):
    """Run with NTFF profiling; returns (results, exec_time_ns, profile)."""
    res = _run(inputs, trace=True)
    return res


# revision 20
# speedup vs baseline: 1.3131x; 1.3131x over previous
"""AttnDecoderGRU step — 8-core Trainium2 Bass kernel.

Sharding (tensor-parallel, 3 AllReduces):
  - GRU: gate-dim sharded. Core k computes gate rows {g*1536 + k*192 .. +192}
    for g in {r,z,n} -> h_newT slice [192, 256].
  - Attention: H-sharded. Core k's scores partial uses its h-slice of h_new
    and enc; AllReduce #1 sums score partials [128s, 256b]. Softmax is
    replicated; context for the core's h-slice is then fully local.
  - Concat proj: contraction(2H)-sharded; AllReduce #2 sums pre-tanh
    partials [1536, 256] (transposed layout).
  - Output proj: vocab-sharded (4000 rows of W_out per core); sharded softmax
    via AllReduce #3 of the per-row exp-sums [256].
Matmul operands are fp16 (fp32 accumulate in PSUM, fp16 gets FWL fast
weight loads and full PE rate); gather/softmax/elementwise/collectives f32.

Measured (8x NC_v3 via axon/PJRT, full-input staging excluded from err):
  probs absmax-rel err 4.0e-3 (l2 6.4e-4), h_new absmax-rel 1.7e-4.
Per-core budget estimate (cost model; NTFF profiling unavailable in this
client): DMA reads ~44MB (~125us floor), PE ~120-150us (outproj 42us,
attention per-batch-column matmuls ~60us, GRU 13us), 3 AllReduces
~90-110us partially exposed => ~250-350us/step. Next optimizations:
(1) split AllReduce #2 by batch halves and overlap with outproj start,
(2) tile_position col-packing (4x) for the 1024 attention LDW+matmul
pairs, (3) prefetch encC during scores phase.
"""

import numpy as np
import ml_dtypes
from contextlib import ExitStack

H, V, B, S, C = 1536, 32000, 256, 128, 8
HK = H // C          # 192  per-core h slice
GK = 3 * HK          # 576  per-core gate rows
VK = V // C          # 4000 per-core vocab slice
P = 128

_CACHE = {}


def _build_nc():
    import concourse.bass as bass
    import concourse.tile as tile
    import concourse.mybir as mybir
    from concourse import bacc
    from concourse.masks import make_identity

    dt = mybir.dt
    F32, BF, I32 = dt.float32, dt.bfloat16, dt.int32
    FR = dt.float32r
    F16 = dt.float16
    AF = mybir.ActivationFunctionType
    Alu = mybir.AluOpType
    AX = mybir.AxisListType

    nc = bacc.Bacc("TRN2", target_bir_lowering=False, debug=False, num_devices=C)

    # ---- I/O ----
    ids_p = nc.declare_dram_parameter("ids32", [B], I32, isOutput=False)
    emb_p = nc.declare_dram_parameter("emb", [V, H], F32, isOutput=False)
    hk_p = nc.declare_dram_parameter("hk32", [HK, B], F32, isOutput=False)
    hT_p = nc.declare_dram_parameter("hT32", [H, B], dt.float16, isOutput=False)
    wih_p = nc.declare_dram_parameter("wih_t", [H, GK], dt.float16, isOutput=False)
    whh_p = nc.declare_dram_parameter("whh_t", [H, GK], dt.float16, isOutput=False)
    brz_p = nc.declare_dram_parameter("brz", [2 * HK], F32, isOutput=False)
    bin_p = nc.declare_dram_parameter("bin_", [HK], F32, isOutput=False)
    bhn_p = nc.declare_dram_parameter("bhn", [HK], F32, isOutput=False)
    encA_p = nc.declare_dram_parameter("encA", [HK, B, S], dt.float16, isOutput=False)  # [h,b,s]
    encC_p = nc.declare_dram_parameter("encC", [S, B, HK], dt.float16, isOutput=False)  # [s,b,h]
    wc_p = nc.declare_dram_parameter("wc_t", [2 * HK, H], dt.float16, isOutput=False)
    bc_p = nc.declare_dram_parameter("bc", [H], F32, isOutput=False)
    wo_p = nc.declare_dram_parameter("wo_t", [H, VK], dt.float16, isOutput=False)
    bo_p = nc.declare_dram_parameter("bo32", [VK], dt.float16, isOutput=False)
    hout_p = nc.declare_dram_parameter("h_out", [HK, B], F32, isOutput=True)
    probs_p = nc.declare_dram_parameter("probs", [B, VK], F32, isOutput=True)

    RG = [list(range(C))]
    NHC = H // P  # 12

    with tile.TileContext(nc) as tc, ExitStack() as ctx:
        sing = ctx.enter_context(tc.tile_pool(name="sing", bufs=1))
        dramp = ctx.enter_context(tc.tile_pool(name="dram", bufs=1, space="DRAM"))

        gruw_cm = tc.tile_pool(name="gruw", bufs=1)
        gruw = gruw_cm.__enter__()

        # ---- embedding gather first (indirect DMA must carry few waits) ----
        xgs = []
        for bb in range(2):
            idt = gruw.tile([P, 1], I32, tag=f"idt{bb}", name=f"idt{bb}")
            nc.gpsimd.dma_start(
                out=idt[:], in_=ids_p[bb * P:(bb + 1) * P].rearrange("(p o) -> p o", o=1))
            xg = gruw.tile([P, H], F32, tag=f"xg{bb}", name=f"xg{bb}")
            nc.gpsimd.indirect_dma_start(
                out=xg[:], out_offset=None, in_=emb_p[:, :],
                in_offset=bass.IndirectOffsetOnAxis(ap=idt[:, :1], axis=0))
            xgs.append(xg)

        ident = sing.tile([P, P], F32, tag="ident")
        make_identity(nc, ident[:])

        # persistent loads
        hTr = gruw.tile([P, NHC, B], F16, tag="hTr")
        nc.gpsimd.dma_start(out=hTr[:], in_=hT_p.rearrange("(c p) b -> p c b", p=P))
        wih_s = gruw.tile([P, NHC, GK], F16, tag="wih")
        nc.gpsimd.dma_start(out=wih_s[:], in_=wih_p.rearrange("(c p) g -> p c g", p=P))
        whh_s = gruw.tile([P, NHC, GK], F16, tag="whh")
        nc.gpsimd.dma_start(out=whh_s[:], in_=whh_p.rearrange("(c p) g -> p c g", p=P))
        hk0 = sing.tile([P, B], F32, tag="hk0")
        nc.gpsimd.dma_start(out=hk0[:], in_=hk_p[0:P, :])
        hk1 = sing.tile([HK - P, B], F32, tag="hk1")
        nc.gpsimd.dma_start(out=hk1[:], in_=hk_p[P:HK, :])
        bc_s = sing.tile([P, NHC], F32, tag="bc")
        nc.gpsimd.dma_start(out=bc_s[:], in_=bc_p.rearrange("(c p) -> p c", p=P))

        # gate biases: rz chunks aligned per gate: r:(0,128),(128,64) z:(192,128),(320,64)
        rz_chunks = [(0, P), (P, HK - P), (HK, P), (HK + P, HK - P)]
        brz_t = []
        for i, (off, cnt) in enumerate(rz_chunks):
            t = sing.tile([cnt, 1], F32, tag=f"brz{i}")
            nc.gpsimd.dma_start(out=t[:], in_=brz_p[off:off + cnt].rearrange("(p o) -> p o", o=1))
            brz_t.append(t)
        n_chunks = [(0, P), (P, HK - P)]  # offsets within n-slice
        bin_t, bhn_t = [], []
        for i, (off, cnt) in enumerate(n_chunks):
            t = sing.tile([cnt, 1], F32, tag=f"bin{i}")
            nc.gpsimd.dma_start(out=t[:], in_=bin_p[off:off + cnt].rearrange("(p o) -> p o", o=1))
            bin_t.append(t)
            t2 = sing.tile([cnt, 1], F32, tag=f"bhn{i}")
            nc.gpsimd.dma_start(out=t2[:], in_=bhn_p[off:off + cnt].rearrange("(p o) -> p o", o=1))
            bhn_t.append(t2)

        # ---- transpose x -> xTr [P, NHC, B] (f32r) ----
        xTr = gruw.tile([P, NHC, B], F16, tag="xTr")
        with tc.tile_pool(name="psT1", bufs=2, space="PSUM") as psT:
            for bb in range(2):
                for hc in range(NHC):
                    tp = psT.tile([P, P], F32, tag="tp")
                    nc.tensor.transpose(out=tp[:], in_=xgs[bb][:, hc * P:(hc + 1) * P], identity=ident[:])
                    nc.vector.tensor_copy(out=xTr[:, hc, bb * P:(bb + 1) * P], in_=tp[:])

        # ---- GRU ----
        gp_cm = tc.tile_pool(name="gp", bufs=2)
        gp = gp_cm.__enter__()
        psG_cm = tc.tile_pool(name="psG", bufs=4, space="PSUM")
        psG = psG_cm.__enter__()
        rz_sb = []
        for i, (off, cnt) in enumerate(rz_chunks):
            ps = psG.tile([cnt, B], F32, tag="gps")
            for hc in range(NHC):
                nc.tensor.matmul(out=ps[:], lhsT=wih_s[:, hc, off:off + cnt], rhs=xTr[:, hc, :],
                                 start=(hc == 0), stop=False)
            for hc in range(NHC):
                nc.tensor.matmul(out=ps[:], lhsT=whh_s[:, hc, off:off + cnt], rhs=hTr[:, hc, :],
                                 start=False, stop=(hc == NHC - 1))
            g = gp.tile([cnt, B], F32, tag=f"rz{i}")
            nc.scalar.activation(out=g[:], in_=ps[:], func=AF.Sigmoid, bias=brz_t[i][:])
            rz_sb.append(g)

        hn_f, hnbf = [], []
        for i, (off, cnt) in enumerate(n_chunks):
            goff = 2 * HK + off
            ps_gi = psG.tile([cnt, B], F32, tag="gps")
            for hc in range(NHC):
                nc.tensor.matmul(out=ps_gi[:], lhsT=wih_s[:, hc, goff:goff + cnt], rhs=xTr[:, hc, :],
                                 start=(hc == 0), stop=(hc == NHC - 1))
            ps_gh = psG.tile([cnt, B], F32, tag="gps")
            for hc in range(NHC):
                nc.tensor.matmul(out=ps_gh[:], lhsT=whh_s[:, hc, goff:goff + cnt], rhs=hTr[:, hc, :],
                                 start=(hc == 0), stop=(hc == NHC - 1))
            ghn = gp.tile([cnt, B], F32, tag="ghn")
            nc.scalar.activation(out=ghn[:], in_=ps_gh[:], func=AF.Identity, bias=bhn_t[i][:])
            t1 = gp.tile([cnt, B], F32, tag="t1")
            nc.vector.tensor_mul(out=t1[:], in0=rz_sb[i][:], in1=ghn[:])  # r * (ghn+bhn)
            t2 = gp.tile([cnt, B], F32, tag="t2")
            nc.vector.tensor_add(out=t2[:], in0=t1[:], in1=ps_gi[:])
            n_t = gp.tile([cnt, B], F32, tag="nt")
            nc.scalar.activation(out=n_t[:], in_=t2[:], func=AF.Tanh, bias=bin_t[i][:])
            hk_c = hk0 if i == 0 else hk1
            d = gp.tile([cnt, B], F32, tag="d")
            nc.vector.tensor_sub(out=d[:], in0=hk_c[:], in1=n_t[:])
            zd = gp.tile([cnt, B], F32, tag="zd")
            nc.vector.tensor_mul(out=zd[:], in0=rz_sb[2 + i][:], in1=d[:])
            hn = sing.tile([cnt, B], F32, tag=f"hn{i}", name=f"hn{i}")
            nc.vector.tensor_add(out=hn[:], in0=n_t[:], in1=zd[:])
            hn_f.append(hn)
            hb = sing.tile([cnt, B], F16, tag=f"hnbf{i}")
            nc.vector.tensor_copy(out=hb[:], in_=hn[:])
            hnbf.append(hb)
            nc.gpsimd.dma_start(out=hout_p[off:off + cnt, :], in_=hn[:])

        psG_cm.__exit__(None, None, None)
        gp_cm.__exit__(None, None, None)
        gruw_cm.__exit__(None, None, None)

        # ---- preload all W_out tiles (overlaps attention + AllReduces) ----
        nts = [(i * 512, 512) for i in range(7)] + [(3584, VK - 3584)]
        wop = ctx.enter_context(tc.tile_pool(name="wop", bufs=1))
        wo_all = wop.tile([P, len(nts) * NHC, 512], F16, tag="wo_all")
        for ni, (noff, ncnt) in enumerate(nts):
            for hc in range(NHC):
                eng = [nc.scalar, nc.gpsimd][(ni * NHC + hc) % 2]
                eng.dma_start(out=wo_all[:, ni * NHC + hc, :ncnt],
                              in_=wo_p[hc * P:(hc + 1) * P, noff:noff + ncnt])
        bo_all = wop.tile([1, VK], F16, tag="bo_all")
        nc.gpsimd.dma_start(out=bo_all[:], in_=bo_p.rearrange("(o n) -> o n", o=1))

        # ---- scores (partial over this core's h-slice) ----
        psS_cm = tc.tile_pool(name="psS", bufs=1, space="PSUM")
        psS = psS_cm.__enter__()
        scps = psS.tile([P, B], F32, tag="scps")
        with tc.tile_pool(name="apool", bufs=2) as apool:
            for blk in range(B // 32):
                b0 = blk * 32
                eA0 = apool.tile([P, 32, S], F16, tag="eA0")
                nc.scalar.dma_start(out=eA0[:], in_=encA_p[0:P, b0:b0 + 32, :])
                eA1 = apool.tile([HK - P, 32, S], F16, tag="eA1")
                nc.gpsimd.dma_start(out=eA1[:], in_=encA_p[P:HK, b0:b0 + 32, :])
                for b in range(32):
                    ba = b0 + b
                    nc.tensor.matmul(out=scps[:, ba:ba + 1], lhsT=eA0[:, b, :],
                                     rhs=hnbf[0][:, ba:ba + 1], start=True, stop=False)
                    nc.tensor.matmul(out=scps[:, ba:ba + 1], lhsT=eA1[:, b, :],
                                     rhs=hnbf[1][:, ba:ba + 1], start=False, stop=True)
        sc_sb = sing.tile([P, B], F32, tag="sc_sb")
        nc.vector.tensor_copy(out=sc_sb[:], in_=scps[:])
        sc_in = dramp.tile([P, B], F32, tag="sc_in")
        nc.gpsimd.dma_start(out=sc_in[:], in_=sc_sb[:])
        sc_out = dramp.tile([P, B], F32, tag="sc_out", addr_space="Shared")
        nc.gpsimd.collective_compute(
            "AllReduce", Alu.add, replica_groups=RG, ins=[sc_in[:].opt()], outs=[sc_out[:].opt()])
        scf = sing.tile([P, B], F32, tag="scf")
        nc.gpsimd.dma_start(out=scf[:], in_=sc_out[:])

        # ---- softmax over s (free dim after transpose) ----
        attbf = sing.tile([P, B], F16, tag="attbf")  # [s, b]
        with tc.tile_pool(name="smx", bufs=4) as smx, \
             tc.tile_pool(name="psT2", bufs=2, space="PSUM") as psT:
            for bb in range(2):
                tp = psT.tile([P, P], F32, tag="tp")
                nc.tensor.transpose(out=tp[:], in_=scf[:, bb * P:(bb + 1) * P], identity=ident[:])
                scb = smx.tile([P, P], F32, tag="scb")
                nc.vector.tensor_copy(out=scb[:], in_=tp[:])
                mx = smx.tile([P, 1], F32, tag="mx")
                nc.vector.reduce_max(out=mx[:], in_=scb[:], axis=AX.X)
                nmx = smx.tile([P, 1], F32, tag="nmx")
                nc.vector.tensor_scalar_mul(nmx[:], scb_in0 := mx[:], -1.0)
                ex = smx.tile([P, P], F32, tag="ex")
                nc.scalar.activation(out=ex[:], in_=scb[:], func=AF.Exp, bias=nmx[:])
                sm = smx.tile([P, 1], F32, tag="sm")
                nc.vector.reduce_sum(out=sm[:], in_=ex[:], axis=AX.X)
                ri = smx.tile([P, 1], F32, tag="ri")
                nc.vector.reciprocal(out=ri[:], in_=sm[:])
                at = smx.tile([P, P], F32, tag="at")
                nc.vector.tensor_scalar_mul(at[:], ex[:], ri[:])
                tp2 = psT.tile([P, P], F32, tag="tp")
                nc.tensor.transpose(out=tp2[:], in_=at[:], identity=ident[:])
                nc.vector.tensor_copy(out=attbf[:, bb * P:(bb + 1) * P], in_=tp2[:])

        # ---- context (local, h-sliced) ----
        cx0 = psS.tile([P, B], F32, tag="cx0")
        cx1 = psS.tile([HK - P, B], F32, tag="cx1")
        with tc.tile_pool(name="cpool", bufs=2) as cpool:
            for blk in range(B // 32):
                b0 = blk * 32
                eC = cpool.tile([S, 32, HK], F16, tag="eC")
                (nc.gpsimd if blk % 2 == 0 else nc.scalar).dma_start(
                    out=eC[:], in_=encC_p[:, b0:b0 + 32, :])
                for b in range(32):
                    ba = b0 + b
                    nc.tensor.matmul(out=cx0[:, ba:ba + 1], lhsT=eC[:, b, 0:P],
                                     rhs=attbf[:, ba:ba + 1], start=True, stop=True)
                    nc.tensor.matmul(out=cx1[:, ba:ba + 1], lhsT=eC[:, b, P:HK],
                                     rhs=attbf[:, ba:ba + 1], start=True, stop=True)
        ctb0 = sing.tile([P, B], F16, tag="ctb0")
        nc.vector.tensor_copy(out=ctb0[:], in_=cx0[:])
        ctb1 = sing.tile([HK - P, B], F16, tag="ctb1")
        nc.vector.tensor_copy(out=ctb1[:], in_=cx1[:])
        psS_cm.__exit__(None, None, None)

        # cat rows: [hn(192); ctx(192)] -> 3 chunks of 128 (partition-shift via DMA)
        cat0 = sing.tile([P, B], F16, tag="cat0")
        nc.vector.tensor_copy(out=cat0[:], in_=hn_f[0][:])
        cat1 = sing.tile([P, B], F16, tag="cat1")
        nc.vector.tensor_copy(out=cat1[0:HK - P, :], in_=hn_f[1][:])
        nc.gpsimd.dma_start(out=cat1[HK - P:P, :], in_=ctb0[0:2 * P - HK, :])
        cat2 = sing.tile([P, B], F16, tag="cat2")
        nc.gpsimd.dma_start(out=cat2[0:HK - P, :], in_=ctb0[2 * P - HK:P, :])
        nc.gpsimd.dma_start(out=cat2[HK - P:P, :], in_=ctb1[:])
        cats = [cat0, cat1, cat2]

        # ---- concat proj (partial) + AllReduce + tanh -> cobf chunks ----
        cp_in = dramp.tile([NHC, P, B], F32, tag="cp_in")
        cp_out = dramp.tile([NHC, P, B], F32, tag="cp_out", addr_space="Shared")
        with tc.tile_pool(name="cc", bufs=4) as cc, \
             tc.tile_pool(name="wcp", bufs=1) as wcp, \
             tc.tile_pool(name="psC", bufs=4, space="PSUM") as psC:
            wc_s = wcp.tile([P, 3, H], F16, tag="wc")
            nc.gpsimd.dma_start(out=wc_s[:], in_=wc_p.rearrange("(c p) m -> p c m", p=P))
            for mc in range(NHC):
                cps = psC.tile([P, B], F32, tag="cps")
                for kc in range(3):
                    nc.tensor.matmul(out=cps[:], lhsT=wc_s[:, kc, mc * P:(mc + 1) * P],
                                     rhs=cats[kc][:], start=(kc == 0), stop=(kc == 2))
                csb = cc.tile([P, B], F32, tag="csb")
                nc.vector.tensor_copy(out=csb[:], in_=cps[:])
                nc.gpsimd.dma_start(out=cp_in[mc], in_=csb[:])
        nc.gpsimd.collective_compute(
            "AllReduce", Alu.add, replica_groups=RG, ins=[cp_in[:].opt()], outs=[cp_out[:].opt()])
        cobf = []
        with tc.tile_pool(name="co", bufs=4) as co:
            for mc in range(NHC):
                cof = co.tile([P, B], F32, tag="cof")
                nc.gpsimd.dma_start(out=cof[:], in_=cp_out[mc])
                cb = sing.tile([P, B], F16, tag=f"cobf{mc}", name=f"cobf{mc}")
                nc.scalar.activation(out=cb[:], in_=cof[:], func=AF.Tanh, bias=bc_s[:, mc:mc + 1])
                cobf.append(cb)

        # ---- output projection + exp (vocab shard) ----
        ones_f = sing.tile([1, P], F32, tag="ones_f")
        nc.vector.memset(ones_f[:], 1.0)
        ones1 = sing.tile([1, P], F16, tag="ones1")
        nc.vector.tensor_copy(out=ones1[:], in_=ones_f[:])
        dnt = [sing.tile([P, len(nts)], F32, tag=f"dnt{bb}", name=f"dnt{bb}") for bb in range(2)]
        ex_tiles = {}
        expool = ctx.enter_context(tc.tile_pool(name="expool", bufs=2 * len(nts)))
        with tc.tile_pool(name="psO", bufs=4, space="PSUM") as psO:
            for ni, (noff, ncnt) in enumerate(nts):
                for bb in range(2):
                    lg = psO.tile([P, 512], F32, tag="lg")
                    nc.tensor.matmul(out=lg[:, :ncnt], lhsT=ones1[:], rhs=bo_all[:, noff:noff + ncnt],
                                     start=True, stop=False)
                    for hc in range(NHC):
                        nc.tensor.matmul(out=lg[:, :ncnt], lhsT=cobf[hc][:, bb * P:(bb + 1) * P],
                                         rhs=wo_all[:, ni * NHC + hc, :ncnt], start=False, stop=(hc == NHC - 1))
                    ex = expool.tile([P, 512], F32, tag="ex")
                    nc.scalar.activation(out=ex[:, :ncnt], in_=lg[:, :ncnt], func=AF.Exp,
                                         accum_out=dnt[bb][:, ni:ni + 1])
                    ex_tiles[(bb, ni)] = ex

        # ---- sharded softmax denom: AllReduce, then scale + write out ----
        dn_in = dramp.tile([2, P], F32, tag="dn_in")
        dn_out = dramp.tile([2, P], F32, tag="dn_out", addr_space="Shared")
        with tc.tile_pool(name="dn", bufs=4) as dnp:
            for bb in range(2):
                ds = dnp.tile([P, 1], F32, tag="ds")
                nc.vector.reduce_sum(out=ds[:], in_=dnt[bb][:], axis=AX.X)
                nc.gpsimd.dma_start(out=dn_in[bb], in_=ds[:])
            nc.gpsimd.collective_compute(
                "AllReduce", Alu.add, replica_groups=RG, ins=[dn_in[:].opt()], outs=[dn_out[:].opt()])
            for bb in range(2):
                dr = dnp.tile([P, 1], F32, tag="dr")
                nc.gpsimd.dma_start(out=dr[:], in_=dn_out[bb])
                ri = dnp.tile([P, 1], F32, tag="rib")
                nc.vector.reciprocal(out=ri[:], in_=dr[:])
                for ni, (noff, ncnt) in enumerate(nts):
                    ex = ex_tiles[(bb, ni)]
                    nc.vector.tensor_scalar_mul(ex[:, :ncnt], ex[:, :ncnt], ri[:])
                    nc.gpsimd.dma_start(out=probs_p[bb * P:(bb + 1) * P, noff:noff + ncnt],
                                        in_=ex[:, :ncnt])
    if not nc.is_finalized():
        nc.finalize()
    return nc


def _prep_inputs(input_step, last_hidden, encoder_outputs, emb_table,
                 W_ih, W_hh, b_ih, b_hh, W_concat, b_concat, W_out, b_out):
    bf16 = ml_dtypes.bfloat16
    ids32 = np.ascontiguousarray(input_step[0].astype(np.int32))
    hT = np.ascontiguousarray(last_hidden[0].T.astype(np.float32))          # [H, B]
    enc = encoder_outputs.astype(np.float32)                                # [S, B, H]
    encA_full = np.ascontiguousarray(enc.transpose(2, 1, 0)).astype(np.float16)  # [H, B, S]
    enc_bf = enc.astype(np.float16)                                         # [S, B, H]
    emb = np.ascontiguousarray(emb_table.astype(np.float32))
    bsum = (b_ih + b_hh).astype(np.float32)

    in_maps = []
    for k in range(C):
        hs = slice(k * HK, (k + 1) * HK)
        gate_rows = np.concatenate([np.arange(g * H + k * HK, g * H + (k + 1) * HK) for g in range(3)])
        wih_k = np.ascontiguousarray(W_ih[gate_rows].T.astype(np.float16))  # [H, GK]
        whh_k = np.ascontiguousarray(W_hh[gate_rows].T.astype(np.float16))
        brz = np.ascontiguousarray(bsum[gate_rows[:2 * HK]])                # r,z slices
        bin_ = np.ascontiguousarray(b_ih[2 * H + k * HK:2 * H + (k + 1) * HK].astype(np.float32))
        bhn = np.ascontiguousarray(b_hh[2 * H + k * HK:2 * H + (k + 1) * HK].astype(np.float32))
        ccols = np.concatenate([np.arange(k * HK, (k + 1) * HK), np.arange(H + k * HK, H + (k + 1) * HK)])
        wc_k = np.ascontiguousarray(W_concat[:, ccols].T.astype(np.float16))    # [2HK, H]
        wo_k = np.ascontiguousarray(W_out[k * VK:(k + 1) * VK].T.astype(np.float16))  # [H, VK]
        in_maps.append({
            "ids32": ids32,
            "emb": emb,
            "hk32": np.ascontiguousarray(hT[hs]),
            "hT32": hT.astype(np.float16),
            "wih_t": wih_k,
            "whh_t": whh_k,
            "brz": brz,
            "bin_": bin_,
            "bhn": bhn,
            "encA": np.ascontiguousarray(encA_full[hs]),                    # [HK, B, S]
            "encC": np.ascontiguousarray(enc_bf[:, :, hs]),                 # [S, B, HK]
            "wc_t": wc_k,
            "bc": b_concat.astype(np.float32),
            "wo_t": wo_k,
            "bo32": b_out[k * VK:(k + 1) * VK].astype(np.float16),
        })
    return in_maps


def kernel(**inputs):
    res = _run(inputs, trace=False)
    probs = np.concatenate([res.results[k]["probs"] for k in range(C)], axis=1)  # [B, V]
    hT_new = np.concatenate([res.results[k]["h_out"] for k in range(C)], axis=0)  # [H, B]
    h_new = np.ascontiguousarray(hT_new.T)[None]                                  # [1, B, H]
    return probs.astype(np.float32), h_new.astype(np.float32)


def _run(inputs, trace=False):
    from concourse.bass_utils import run_bass_kernel_spmd
    inputs = {k: np.asarray(v) for k, v in inputs.items()}
    in_maps = _prep_inputs(**inputs)
    if "nc" not in _CACHE:
        _CACHE["nc"] = _build_nc()
    return run_bass_kernel_spmd(_CACHE["nc"], in_maps, list(range(C)), trace=trace)


def kernel_traced(**inputs):
    """Run with NTFF profiling; returns (results, exec_time_ns, profile)."""
    res = _run(inputs, trace=True)
    return res
